# revision 1
# baseline (speedup 1.0000x reference)
"""Multi-head self-attention (B=2, S=4096, D=512, H=8, Dh=64) on 8 TRN2 cores.

Sharding: core i handles batch b = i//4 and head-pair hp = i%4 (heads 2*hp,
2*hp+1).  Each core computes Q/K/V projections for its two heads, flash-style
attention (no-max softmax; scores range is +-9 so exp is safe), and a partial
out-projection.  Host sums the 4 partial outputs per batch and transposes back.

All device tensors use transposed layouts (feature dim on partitions) so every
matmul has its contraction dim on the partition axis with no on-device
transposes:
  xt  [512, S]  = X[b].T
  wq/wk/wv [512, 128] = W[:, hp*128:(hp+1)*128]
  wo  [128, 512] = Wo[hp*128:(hp+1)*128, :]
  yt  [512, S]  = partial (Y[b]).T

Matmuls run as float32r (single-pass PE mode, 1 cycle/row vs 4 for fp32).

TRN2 quirk: an fp32/f32r matmul self-loads weights and its S3_LW slot encodes
exactly ONE sync wait; walrus cannot legalize more on a Matmult ("Too many
sync wait commands").  `_legalize_matmul_waits` post-processes the scheduled
module: extra waits move onto injected single-wait PE no-ops placed directly
before the matmul in its block — semantically identical, walrus-legal.
"""

import sys
from contextlib import ExitStack

for _p in ("/opt/trn_rl_repo",):
    if _p not in sys.path:
        sys.path.insert(0, _p)

import numpy as np

import concourse.bass as bass
import concourse.tile as tile
from concourse import mybir
from concourse.bass_utils import run_bass_kernel_spmd

F32 = mybir.dt.float32
F32R = mybir.dt.float32r
MM_DT = F32R     # single-pass PE mode: 1 cycle/row vs 4 for full fp32
D = 512          # model dim
DH = 64          # head dim
P = 128          # partitions
B = 2
H = 8
S_FULL = 4096
N_CORES = 8
NC_T = D // P    # 4 contraction tiles over model dim

LAST_RESULTS = None  # test harness reads exec_time_ns from here


def _emit(nc: bass.Bass, tc: "tile.TileContext", ctx: ExitStack, S: int):
    """Emit the per-core program. Parameterized by S for small-sim testing."""
    NS = S // 512            # 512-wide seq blocks
    NK = S // P              # 128-row key tiles
    QB = 1024 if S >= 1024 else S
    NQB = S // QB            # scores q-blocks
    QH = QB // 512           # 512-wide halves per q-block
    inv_scale = 1.0 / np.sqrt(DH)

    def mm(out, lhsT, rhs, start=True, stop=True):
        return nc.tensor.matmul(out, lhsT, rhs, start=start, stop=stop)

    xt = nc.declare_dram_parameter("xt", [D, S], MM_DT, isOutput=False)
    wq = nc.declare_dram_parameter("wq", [D, P], MM_DT, isOutput=False)
    wk = nc.declare_dram_parameter("wk", [D, P], MM_DT, isOutput=False)
    wv = nc.declare_dram_parameter("wv", [D, P], MM_DT, isOutput=False)
    wo = nc.declare_dram_parameter("wo", [P, D], MM_DT, isOutput=False)
    yt = nc.declare_dram_parameter("yt", [D, S], F32, isOutput=True)

    const = ctx.enter_context(tc.tile_pool(name="const", bufs=1))

    # ---- load inputs straight to SBUF ----
    xt_sb = []
    for c in range(NC_T):
        t = const.tile([P, S], MM_DT, tag=f"xt{c}", name=f"xt{c}")
        nc.sync.dma_start(out=t[:], in_=xt[c * P:(c + 1) * P, :])
        xt_sb.append(t)
    w_sb = {}
    for name, ap in (("wq", wq), ("wk", wk), ("wv", wv)):
        tiles = []
        for c in range(NC_T):
            t = const.tile([P, P], MM_DT, tag=f"{name}{c}", name=f"{name}{c}")
            nc.sync.dma_start(out=t[:], in_=ap[c * P:(c + 1) * P, :])
            tiles.append(t)
        w_sb[name] = tiles
    wo_sb = const.tile([P, D], MM_DT, tag="wo")
    nc.sync.dma_start(out=wo_sb[:], in_=wo[:, :])

    # persistent intermediates
    qt_sb = const.tile([P, S], MM_DT, tag="qt")      # [2*64 d, S] stacked heads
    kt_sb = const.tile([P, S], MM_DT, tag="kt")
    # V with a ones column appended per k-tile: [128 k, NK*65]; col 64 == 1.0
    vones = [const.tile([P, NK * (DH + 1)], MM_DT, tag=f"vones{h}", name=f"vones{h}")
             for h in range(2)]
    # column 64 of each 65-wide block must be 1.0; memset can't write f32r,
    # but a DVE copy can (it rounds on output)
    konst = const.tile([P, NK, 1], F32, tag="konst")
    nc.vector.memset(konst[:], 1.0)
    for h in range(2):
        vv = vones[h].rearrange("p (k c) -> p k c", c=DH + 1)
        nc.vector.tensor_copy(vv[:, :, DH:DH + 1], konst[:])
    ctx_sb = const.tile([P, S], MM_DT, tag="ctx")    # context^T, stacked heads

    # single PSUM pool: tag "s" (3 x [128,QB]) + tag "ctx" (1 x [65,QB]) = 8 banks
    ps = ctx.enter_context(tc.tile_pool(name="ps", bufs=3, space="PSUM"))
    es = ctx.enter_context(tc.tile_pool(name="es", bufs=3))
    bcp = ctx.enter_context(tc.tile_pool(name="bcp", bufs=2))
    rtp = ctx.enter_context(tc.tile_pool(name="rtp", bufs=2))
    rdp = ctx.enter_context(tc.tile_pool(name="rdp", bufs=2, space="DRAM"))

    # ---- phase A: V first, then Q/K ----
    for k in range(NK):
        ksl = slice(k * P, (k + 1) * P)
        pv = ps.tile([P, P], F32, tag="s", name="pv")
        for c in range(NC_T):
            mm(pv[:], xt_sb[c][:, ksl], w_sb["wv"][c][:],
               start=(c == 0), stop=(c == NC_T - 1))
        for h in range(2):
            nc.vector.tensor_copy(
                vones[h][:, k * (DH + 1):k * (DH + 1) + DH],
                pv[:, h * DH:(h + 1) * DH])
    for qb in range(NS):
        sl = slice(qb * 512, (qb + 1) * 512)
        pq = ps.tile([P, 512], F32, tag="s", name="pq")
        for c in range(NC_T):
            mm(pq[:], w_sb["wq"][c][:], xt_sb[c][:, sl],
               start=(c == 0), stop=(c == NC_T - 1))
        nc.vector.tensor_copy(qt_sb[:, sl], pq[:])
        pk = ps.tile([P, 512], F32, tag="s", name="pk")
        for c in range(NC_T):
            mm(pk[:], w_sb["wk"][c][:], xt_sb[c][:, sl],
               start=(c == 0), stop=(c == NC_T - 1))
        nc.vector.tensor_copy(kt_sb[:, sl], pk[:])

    # ---- phase B: attention (flash, no-max softmax), fused normalize ----
    for h in range(2):
        hsl = slice(h * DH, (h + 1) * DH)
        for qb in range(NQB):
            qsl = slice(qb * QB, (qb + 1) * QB)
            ctx_ps = ps.tile([DH + 1, QB], F32, tag="ctx", bufs=1, name="ctx_ps")
            for k in range(NK):
                s_ps = ps.tile([P, QB], F32, tag="s", name="s_ps")
                lhs_k = kt_sb[hsl, k * P:(k + 1) * P]
                for j in range(QH):
                    jsl = slice(qb * QB + j * 512, qb * QB + (j + 1) * 512)
                    mm(s_ps[:, j * 512:(j + 1) * 512], lhs_k, qt_sb[hsl, jsl])
                e_sb = es.tile([P, QB], MM_DT, tag="e", name="e_sb")
                nc.scalar.activation(e_sb[:], s_ps[:],
                                     mybir.ActivationFunctionType.Exp,
                                     scale=inv_scale)
                vo = vones[h][:, k * (DH + 1):(k + 1) * (DH + 1)]
                for j in range(QH):
                    mm(ctx_ps[:, j * 512:(j + 1) * 512], vo,
                       e_sb[:, j * 512:(j + 1) * 512],
                       start=(k == 0), stop=(k == NK - 1))
            # drain + normalize this block
            rt = rtp.tile([1, QB], F32, tag="rt", name="rt")
            nc.vector.tensor_copy(rt[0:1, :], ctx_ps[DH:DH + 1, :])
            nc.vector.reciprocal(rt[0:1, :], rt[0:1, :])
            bc = bcp.tile([DH, QB], F32, tag="bc", name="bc")
            # partition-broadcast rt row 0 to 64 partitions: bounce through
            # DRAM, whose APs allow a stride-0 partition dim (SBUF APs don't)
            rtd = rdp.tile([1, QB], F32, tag="rtd", name="rtd")
            nc.sync.dma_start(out=rtd[:], in_=rt[0:1, :])
            rtd_bcast = bass.AP(tensor=rtd.tensor, offset=rtd.offset,
                                ap=[[0, DH]] + list(rtd[0:1, :].ap)[1:])
            nc.sync.dma_start(out=bc[:], in_=rtd_bcast)
            nc.vector.tensor_mul(ctx_sb[hsl, qsl], ctx_ps[:DH, :], bc[:])

    # ---- phase C: out-projection (partial; host sums across cores) ----
    with tc.tile_pool(name="osb", bufs=2) as osb:
        for e in range(NC_T):
            for sb in range(NS):
                sl = slice(sb * 512, (sb + 1) * 512)
                o_ps = ps.tile([P, 512], F32, tag="s", name="o_ps")
                mm(o_ps[:], wo_sb[:, e * P:(e + 1) * P], ctx_sb[:, sl])
                o_sb = osb.tile([P, 512], F32, tag="osb", name="o_sb")
                nc.scalar.copy(o_sb[:], o_ps[:])
                nc.sync.dma_start(out=yt[e * P:(e + 1) * P, sl], in_=o_sb[:])


_TPB_ENGINES = {mybir.EngineType.PE, mybir.EngineType.Activation,
                mybir.EngineType.DVE, mybir.EngineType.Pool}


def _legalize_matmul_waits(nc: bass.Bass) -> int:
    """Walrus encodes only ONE sync wait on TPB compute instructions (seen on
    Matmult and TensorCopy).  Move extra waits onto injected same-engine
    no-ops (one wait each) placed immediately before the instruction in its
    block: same semantics, legal encoding."""
    n_fixed = 0
    for f in nc.m.functions:
        for bb in f.blocks:
            out = []
            changed = False
            for ins in bb.instructions:
                si = ins.sync_info
                if (getattr(ins, "engine", None) is not None
                        and si is not None and len(si.on_wait) > 1):
                    for idx, w in enumerate(si.on_wait[:-1]):
                        nop = mybir.InstNoOp(name=f"{ins.name}-lgw{idx}",
                                             ins=[], outs=[])
                        nop.engine = ins.engine
                        nop.sync_info = mybir.SyncInfo(on_wait=[w], on_update=[])
                        out.append(nop)
                    ins.sync_info = mybir.SyncInfo(on_wait=[si.on_wait[-1]],
                                                   on_update=si.on_update)
                    n_fixed += 1
                    changed = True
                out.append(ins)
            if changed:
                bb.instructions = out
    return n_fixed


def build(S: int = S_FULL, legalize: bool = False) -> bass.Bass:
    nc = bass.Bass()
    with ExitStack() as ctx:
        if MM_DT == F32R:
            ctx.enter_context(nc.allow_low_precision(
                reason="f32r matmul operands (11-bit mantissa)"))
        tc = ctx.enter_context(tile.TileContext(nc))
        _emit(nc, tc, ctx, S)
    if legalize:
        # only for the walrus/hardware path; CoreSim wants updates on every
        # instruction and doesn't enforce the 1-wait Matmult limit
        _legalize_matmul_waits(nc)
    return nc


_NC_CACHE = {}


def _get_nc(S: int) -> bass.Bass:
    if S not in _NC_CACHE:
        _NC_CACHE[S] = build(S, legalize=True)
    return _NC_CACHE[S]


def _round_f32r(a):
    if MM_DT != F32R:
        return np.ascontiguousarray(a, dtype=np.float32)
    u = np.ascontiguousarray(a, dtype=np.float32).view(np.uint32)
    r = (u + 0x7FF + ((u >> 12) & 1)) & np.uint32(0xFFFFF000)
    return r.view(np.float32)


def make_in_maps(X, Wq, Wk, Wv, Wo):
    xts = [_round_f32r(X[b].T) for b in range(B)]
    in_maps = []
    for i in range(N_CORES):
        b, hp = divmod(i, 4)  # 4 head-pairs per batch
        csl = slice(hp * P, (hp + 1) * P)
        in_maps.append({
            "xt": xts[b],
            "wq": _round_f32r(Wq[:, csl]),
            "wk": _round_f32r(Wk[:, csl]),
            "wv": _round_f32r(Wv[:, csl]),
            "wo": _round_f32r(Wo[csl, :]),
        })
    return in_maps


def kernel(X, Wq, Wk, Wv, Wo, _trace=False):
    global LAST_RESULTS
    X = np.asarray(X, dtype=np.float32)
    S = X.shape[1]
    nc = _get_nc(S)
    in_maps = make_in_maps(X, np.asarray(Wq, np.float32), np.asarray(Wk, np.float32),
                           np.asarray(Wv, np.float32), np.asarray(Wo, np.float32))
    res = run_bass_kernel_spmd(nc, in_maps, list(range(N_CORES)), trace=_trace)
    LAST_RESULTS = res
    Y = np.zeros((B, S, D), dtype=np.float32)
    for i in range(N_CORES):
        Y[i // 4] += res.results[i]["yt"].T
    return Y



# revision 6
# speedup vs baseline: 1.5821x; 1.5821x over previous
"""Multi-head self-attention (B=2, S=4096, D=512, H=8, Dh=64) on 8 TRN2 cores.

Sharding: core i handles batch b = i//4 and head-pair hp = i%4 (heads 2*hp,
2*hp+1).  Each core computes Q/K/V projections for its two heads, flash-style
attention (no-max softmax; scores range is +-9 so exp is safe), and a partial
out-projection.  Host sums the 4 partial outputs per batch and transposes back.

v2 design (PE/ACT/DVE co-balanced, all-fp16 datapath):
  - All matmul operands fp16 (10-bit mantissa ~ f32r accuracy class).  fp16
    stationary operands get hidden LDWEIGHTS (pull-ahead) + fast weight load.
  - Scores for the two heads are emitted back-to-back with base_partition
    0/64 slices -> tile_position (0,0)/(64,0) row groups -> the PE runs them
    CONCURRENTLY (row tiling), halving scores stream time.
  - exp() alternates between ACT (exact, activation Exp) and DVE (Schraudolph
    int16 bit trick: e^(s/8) bits ~= trunc(1024*log2e*s/8 + C); the 1024*
    log2e/8 factor is folded into Wq host-side so scores arrive pre-scaled).
    One fused [128, 1024] tile covers both heads per k-tile.
  - V with a ones column appended per k-tile ([128, 65] weights); matmul with
    it accumulates context AND the softmax denominator (row 64) in one pass.
  - Normalize: ctx psum -> SBUF via DMA, reciprocal_approx_fast on the sums
    row, 1/sigma broadcast via DRAM stride-0 bounce, one DVE mul per head.
  - Out-projection per q-block; outputs DMA'd straight from PSUM to DRAM.

TRN2 quirk: walrus encodes only ONE sync wait on TPB compute instructions.
`_legalize_matmul_waits` moves extra waits onto injected single-wait no-ops.
"""

import sys
from contextlib import ExitStack

for _p in ("/opt/trn_rl_repo",):
    if _p not in sys.path:
        sys.path.insert(0, _p)

import numpy as np

import concourse.bass as bass
import concourse.tile as tile
from concourse import mybir
from concourse.bass_utils import run_bass_kernel_spmd

F32 = mybir.dt.float32
F16 = mybir.dt.float16
I16 = mybir.dt.int16
D = 512          # model dim
DH = 64          # head dim
P = 128          # partitions
B = 2
H = 8
S_FULL = 4096
N_CORES = 8
NC_T = D // P    # 4 contraction tiles over model dim

# Schraudolph fp16 exp: bits(e^(s/8)) ~= trunc(s_scaled + SCHRAU_C) where
# s_scaled = (1024*log2e/8) * s arrives pre-scaled (folded into Wq).
LAM16 = 1024.0 * np.log2(np.e) / 8.0          # 184.6644...
SCHRAU_C = 15315.75                            # tuned: max rel err +-3.0e-2
ACT_SCALE = float(np.log(2.0) / 1024.0)        # exp(ACT_SCALE * s_scaled)
# k-tiles whose exp runs on DVE (Schraudolph) instead of ACT (exact)
DVE_KMOD = (2, 5, 7)                           # 3 of 8 -> 37.5% on DVE

LAST_RESULTS = None  # test harness reads exec_time_ns from here


def _emit(nc: bass.Bass, tc: "tile.TileContext", ctx: ExitStack, S: int):
    NS = S // 512            # 512-wide seq blocks
    NK = S // P              # 128-row key tiles
    NQB = S // 512           # q blocks

    def mm(out, lhsT, rhs, start=True, stop=True):
        return nc.tensor.matmul(out, lhsT, rhs, start=start, stop=stop)

    xt = nc.declare_dram_parameter("xt", [D, S], F16, isOutput=False)
    wq = nc.declare_dram_parameter("wq", [D, P], F16, isOutput=False)
    wk = nc.declare_dram_parameter("wk", [D, P], F16, isOutput=False)
    wv = nc.declare_dram_parameter("wv", [D, P], F16, isOutput=False)
    wo = nc.declare_dram_parameter("wo", [P, D], F16, isOutput=False)
    yt = nc.declare_dram_parameter("yt", [D, S], F32, isOutput=True)

    const = ctx.enter_context(tc.tile_pool(name="const", bufs=1))

    # ---- weights to SBUF ----
    w_sb = {}
    for name, ap in (("wq", wq), ("wk", wk), ("wv", wv)):
        tiles = []
        for c in range(NC_T):
            t = const.tile([P, P], F16, tag=f"{name}{c}", name=f"{name}{c}")
            nc.sync.dma_start(out=t[:], in_=ap[c * P:(c + 1) * P, :])
            tiles.append(t)
        w_sb[name] = tiles
    wo_sb = const.tile([P, D], F16, tag="wo")
    nc.sync.dma_start(out=wo_sb[:], in_=wo[:, :])

    # ---- xt to SBUF, one tile per (chunk, 512-block) so compute pipelines
    # with the load (j-major order) ----
    xtc = [[None] * NS for _ in range(NC_T)]
    for j in range(NS):
        for c in range(NC_T):
            t = const.tile([P, 512], F16, tag=f"xt{c}_{j}", name=f"xt{c}_{j}")
            nc.sync.dma_start(out=t[:], in_=xt[c * P:(c + 1) * P,
                                              j * 512:(j + 1) * 512])
            xtc[c][j] = t

    # persistent intermediates (all fp16)
    qt_sb = const.tile([P, S], F16, tag="qt")      # [2*64 d, S], pre-scaled
    kt_sb = const.tile([P, S], F16, tag="kt")
    # V with a ones column appended per k-tile: [128 k, NK*65]; col 64 == 1.0
    vones = [const.tile([P, NK * (DH + 1)], F16, tag=f"vones{h}",
                        name=f"vones{h}") for h in range(2)]
    for h in range(2):
        vv = vones[h].rearrange("p (k c) -> p k c", c=DH + 1)
        nc.vector.memset(vv[:, :, DH:DH + 1], 1.0)

    # ---- phase A: projections ----
    with tc.tile_pool(name="pa", bufs=2, space="PSUM") as pa:
        for j in range(NS):
            jsl = slice(j * 512, (j + 1) * 512)
            pq = pa.tile([P, 512], F32, tag="pq", name="pq")
            for c in range(NC_T):
                mm(pq[:], w_sb["wq"][c][:], xtc[c][j][:],
                   start=(c == 0), stop=(c == NC_T - 1))
            nc.vector.tensor_copy(qt_sb[:, jsl], pq[:])
            pk = pa.tile([P, 512], F32, tag="pk", name="pk")
            for c in range(NC_T):
                mm(pk[:], w_sb["wk"][c][:], xtc[c][j][:],
                   start=(c == 0), stop=(c == NC_T - 1))
            nc.scalar.copy(kt_sb[:, jsl], pk[:])
            for t in range(4):
                k = j * 4 + t
                tsl = slice(t * P, (t + 1) * P)
                pv = pa.tile([P, P], F32, tag="pv", name="pv")
                for c in range(NC_T):
                    mm(pv[:], xtc[c][j][:, tsl], w_sb["wv"][c][:],
                       start=(c == 0), stop=(c == NC_T - 1))
                nc.vector.tensor_copy(
                    vones[0][:, k * (DH + 1):k * (DH + 1) + DH], pv[:, 0:DH])
                nc.scalar.copy(
                    vones[1][:, k * (DH + 1):k * (DH + 1) + DH], pv[:, DH:P])

    # ---- phase B: attention + phase C fused per q-block ----
    ps = ctx.enter_context(tc.tile_pool(name="ps", bufs=2, space="PSUM"))
    es = ctx.enter_context(tc.tile_pool(name="es", bufs=3))
    cu = ctx.enter_context(tc.tile_pool(name="cu", bufs=2))
    sv = ctx.enter_context(tc.tile_pool(name="sv", bufs=2))
    bcp = ctx.enter_context(tc.tile_pool(name="bcp", bufs=2))
    csp = ctx.enter_context(tc.tile_pool(name="csp", bufs=2))
    osp = ctx.enter_context(tc.tile_pool(name="osp", bufs=2))
    rdp = ctx.enter_context(tc.tile_pool(name="rdp", bufs=2, space="DRAM"))

    for qb in range(NQB):
        qsl = slice(qb * 512, (qb + 1) * 512)
        # ctx accumulator: h0 in cols 0:512, h1 in cols 512:1024; row 64 = sums
        ctx2 = ps.tile([DH + 1, 1024], F32, tag="ctx", bufs=1, name="ctx2")
        for k in range(NK):
            ksl = slice(k * P, (k + 1) * P)
            sp = ps.tile([P, 1024], F32, tag="s", name="sp")
            # two heads' scores: base_partition 0/64 slices -> row-tiled pair
            for h in range(2):
                hsl = slice(h * DH, (h + 1) * DH)
                mm(sp[:, h * 512:(h + 1) * 512],
                   kt_sb[hsl, ksl], qt_sb[hsl, qsl])
            e = es.tile([P, 1024], I16, tag="e", name="e")
            if (k % 8) in DVE_KMOD:
                # Schraudolph: int16 bits of fp16 e^(s/8)
                nc.vector.tensor_scalar(e[:], sp[:], SCHRAU_C, None,
                                        mybir.AluOpType.add)
            else:
                nc.scalar.activation(e[:].bitcast(F16), sp[:],
                                     mybir.ActivationFunctionType.Exp,
                                     scale=ACT_SCALE)
            ef = e[:].bitcast(F16)
            for h in range(2):
                vo = vones[h][:, k * (DH + 1):(k + 1) * (DH + 1)]
                mm(ctx2[:, h * 512:(h + 1) * 512], vo,
                   ef[:, h * 512:(h + 1) * 512],
                   start=(k == 0), stop=(k == NK - 1))
        # drain ctx (unnormalized) to SBUF, freeing the psum bank (ACT copy)
        ctxU = cu.tile([DH + 1, 1024], F32, tag="cu", name="ctxU")
        nc.scalar.copy(ctxU[:], ctx2[:])
        # 1/sigma: DMA-reshape the sums row to [64, 16] (DVE reciprocal is
        # 8 cyc/elem along free dim -- spread it across partitions), exact
        # reciprocal, then broadcast via DRAM stride-0 bounce
        sg = sv.tile([DH, 16], F32, tag="sg", name="sg")
        nc.sync.dma_start(out=sg[:], in_=ctxU[DH:DH + 1, :])
        sr = sv.tile([DH, 16], F32, tag="sr", name="sr")
        nc.vector.reciprocal(sr[:], sg[:])
        sd = rdp.tile([1, 1024], F32, tag="sd", name="sd")
        nc.sync.dma_start(out=sd[0:1, :], in_=sr[:])
        bc = bcp.tile([DH, 1024], F32, tag="bc", name="bc")
        sd_bcast = bass.AP(tensor=sd.tensor, offset=sd.offset,
                           ap=[[0, DH]] + list(sd[0:1, :].ap)[1:])
        nc.sync.dma_start(out=bc[:], in_=sd_bcast)
        # normalized ctx, fp16, heads stacked on partitions for out-proj
        ctxs = csp.tile([P, 512], F16, tag="ctxs", name="ctxs")
        for h in range(2):
            nc.vector.tensor_mul(ctxs[h * DH:(h + 1) * DH, :],
                                 ctxU[0:DH, h * 512:(h + 1) * 512],
                                 bc[:, h * 512:(h + 1) * 512])
        # phase C for this q-block: 4 output-chunk matmuls in 2 psum pairs;
        # drain pairs via alternating ACT/DVE copies, then DMA out
        for pr in range(2):
            o_ps = ps.tile([P, 1024], F32, tag="o", bufs=1, name="o_ps")
            for i in range(2):
                e4 = pr * 2 + i
                mm(o_ps[:, i * 512:(i + 1) * 512],
                   wo_sb[:, e4 * P:(e4 + 1) * P], ctxs[:])
            o_sb = osp.tile([P, 1024], F32, tag="osb", name="o_sb")
            if pr == 0:
                nc.scalar.copy(o_sb[:], o_ps[:])
            else:
                nc.vector.tensor_copy(o_sb[:], o_ps[:])
            for i in range(2):
                e4 = pr * 2 + i
                nc.sync.dma_start(out=yt[e4 * P:(e4 + 1) * P, qsl],
                                  in_=o_sb[:, i * 512:(i + 1) * 512])


_TPB_ENGINES = {mybir.EngineType.PE, mybir.EngineType.Activation,
                mybir.EngineType.DVE, mybir.EngineType.Pool}


def _legalize_matmul_waits(nc: bass.Bass) -> int:
    """Walrus encodes only ONE sync wait on TPB compute instructions (seen on
    Matmult and TensorCopy).  Move extra waits onto injected same-engine
    no-ops (one wait each) placed immediately before the instruction in its
    block: same semantics, legal encoding."""
    n_fixed = 0
    for f in nc.m.functions:
        for bb in f.blocks:
            out = []
            changed = False
            for ins in bb.instructions:
                si = ins.sync_info
                if (getattr(ins, "engine", None) is not None
                        and si is not None and len(si.on_wait) > 1):
                    for idx, w in enumerate(si.on_wait[:-1]):
                        nop = mybir.InstNoOp(name=f"{ins.name}-lgw{idx}",
                                             ins=[], outs=[])
                        nop.engine = ins.engine
                        nop.sync_info = mybir.SyncInfo(on_wait=[w], on_update=[])
                        out.append(nop)
                    ins.sync_info = mybir.SyncInfo(on_wait=[si.on_wait[-1]],
                                                   on_update=si.on_update)
                    n_fixed += 1
                    changed = True
                out.append(ins)
            if changed:
                bb.instructions = out
    return n_fixed


def build(S: int = S_FULL, legalize: bool = False) -> bass.Bass:
    nc = bass.Bass()
    with ExitStack() as ctx:
        ctx.enter_context(nc.allow_low_precision(
            reason="fp16 matmul operands / int16 exp bit-trick"))
        tc = ctx.enter_context(tile.TileContext(nc))
        _emit(nc, tc, ctx, S)
    if legalize:
        # only for the walrus/hardware path; CoreSim wants updates on every
        # instruction and doesn't enforce the 1-wait Matmult limit
        _legalize_matmul_waits(nc)
    return nc


_NC_CACHE = {}


def _get_nc(S: int) -> bass.Bass:
    if S not in _NC_CACHE:
        _NC_CACHE[S] = build(S, legalize=True)
    return _NC_CACHE[S]


def make_in_maps(X, Wq, Wk, Wv, Wo):
    X = np.asarray(X, np.float32)
    Wq = np.asarray(Wq, np.float32)
    Wk = np.asarray(Wk, np.float32)
    Wv = np.asarray(Wv, np.float32)
    Wo = np.asarray(Wo, np.float32)
    xts = [np.ascontiguousarray(X[b].T).astype(np.float16) for b in range(B)]
    in_maps = []
    for i in range(N_CORES):
        b, hp = divmod(i, 4)  # 4 head-pairs per batch
        csl = slice(hp * P, (hp + 1) * P)
        in_maps.append({
            "xt": xts[b],
            # fold the Schraudolph/ACT pre-scale into Wq
            "wq": np.ascontiguousarray(Wq[:, csl] * LAM16).astype(np.float16),
            "wk": np.ascontiguousarray(Wk[:, csl]).astype(np.float16),
            "wv": np.ascontiguousarray(Wv[:, csl]).astype(np.float16),
            "wo": np.ascontiguousarray(Wo[csl, :]).astype(np.float16),
        })
    return in_maps


def kernel(X, Wq, Wk, Wv, Wo, _trace=False):
    global LAST_RESULTS
    X = np.asarray(X, dtype=np.float32)
    S = X.shape[1]
    nc = _get_nc(S)
    in_maps = make_in_maps(X, Wq, Wk, Wv, Wo)
    res = run_bass_kernel_spmd(nc, in_maps, list(range(N_CORES)), trace=_trace)
    LAST_RESULTS = res
    Y = np.zeros((B, S, D), dtype=np.float32)
    for i in range(N_CORES):
        Y[i // 4] += res.results[i]["yt"].T
    return Y
